# revision 8
# baseline (speedup 1.0000x reference)
"""Trainium2 Bass kernel for Battaglia-style GNN message passing (SPMD, 8 cores).

Problem:
    edge_in = concat(h[src], h[dst], e)            # [E, 144]
    msg     = relu(edge_in @ W_msg + b_msg)        # [E, 64]
    agg     = segment_sum(msg, dst, N)             # [N, 64]
    h_new   = relu(concat(h, agg) @ W_upd + b_upd) # [N, 64]
    phis    = segment_sum(h_new, node2graph, G)    # [G, 64]

Strategy (dst-sharded edge-parallel):
  * Host sorts edges by dst, shards nodes (and their incoming edges)
    contiguously across 8 cores; 128-node blocks; per-block edge runs are
    split by src < HALF (int16 gather-index range) into A/B runs, each
    padded to a uniform chunk count (CA/CB chunks of 128 edges).
  * Algebraic folding: W_msg = [W1; W2; W3] rows for (h_src, h_dst, e).
    Device precomputes P1 = h@W1 (full, to DRAM gather table),
    U = h@W2 + b_msg and Q = h@Wu1 + b_upd for own node shard (SBUF).
  * Per edge chunk [128e]:  psum = S_gT.T @ u_block + eT.T @ W3;
    msg = relu(psum + P1[src]) where P1[src] comes from bulk dma_gather;
    aggT[64, 128n] += msg.T-matmul with one-hot S_g (the scatter).
    One-hots built on DVE via iota/is_equal; S_gT via partition_broadcast.
  * Node update + graph readout per block via small matmuls; per-core
    partial phis summed on host; h_new shards concatenated on host.
"""
import sys

for _p in ("/opt/trn_rl_repo",):
    if _p not in sys.path:
        sys.path.insert(0, _p)

import numpy as np

from concourse import bacc, mybir
import concourse.tile as tile

P = 128
N_NODES = 50000
N_EDGES = 800000
ATOM = 64
BOND = 16
HDIM = 64
NG = 128
NCORES = 8

NB = 392                  # node blocks total (N padded to 50176)
N_PAD = NB * P            # 50176
NBC = NB // NCORES        # 49 blocks per core
NPC = NBC * P             # 6272 nodes per core
HALF = N_PAD // 2         # 25088: int16 gather table split

F32 = mybir.dt.float32
I16 = mybir.dt.int16

# knobs
GG = 4                    # blocks per gather output-tile group
GCALL = 1024              # max idxs per dma_gather call (HW-validated limit)
NBC_RUN = NBC             # blocks actually processed (bisection knob)
FEAT_OFF = set()          # feature-bisection: names to disable

# set by test harness for profiling
RUN_KWARGS = {}


def _ceil_div(a, b):
    return (a + b - 1) // b


def _preprocess(h, e, W_msg, b_msg, W_upd, b_upd, src, dst, node2graph):
    h = np.asarray(h, np.float32)
    e = np.asarray(e, np.float32)
    W_msg = np.asarray(W_msg, np.float32)
    b_msg = np.asarray(b_msg, np.float32)
    W_upd = np.asarray(W_upd, np.float32)
    b_upd = np.asarray(b_upd, np.float32)
    src = np.asarray(src, np.int64)
    dst = np.asarray(dst, np.int64)
    n2g = np.asarray(node2graph, np.int64)

    E = len(src)
    order = np.argsort(dst, kind="stable")
    src_s = src[order]
    dst_s = dst[order]
    e_s = e[order]

    blk = dst_s // P
    isB = (src_s >= HALF).astype(np.int64)
    key = blk * 2 + isB
    order2 = np.argsort(key, kind="stable")
    src2 = src_s[order2]
    dst2 = dst_s[order2]
    e2 = e_s[order2]
    key2 = key[order2]
    blk2 = blk[order2]
    isB2 = isB[order2]

    cnt = np.bincount(key2, minlength=2 * NB)
    CA = max(1, _ceil_div(int(cnt[0::2].max()), P))
    CB = max(1, _ceil_div(int(cnt[1::2].max()), P))
    CC = CA + CB
    E_BLK = CC * P

    starts = np.zeros(2 * NB, np.int64)
    starts[1:] = np.cumsum(cnt)[:-1]
    pos = np.arange(E, dtype=np.int64) - starts[key2]

    core = blk2 // NBC
    bl = blk2 % NBC                    # block within core
    slot = bl * E_BLK + isB2 * (CA * P) + pos   # slot within core edge array
    dstoff = (dst2 - blk2 * P).astype(np.float32)

    LA, LB = NBC * CA * P, NBC * CB * P

    eT = np.zeros((NCORES, BOND, NBC * E_BLK), np.float32)
    eT[core, :, slot] = e2                      # advanced indexing: [E, BOND]
    dstoff_row = np.full((NCORES, NBC, E_BLK), 200.0, np.float32)
    dstoff_row[core, bl, isB2 * (CA * P) + pos] = dstoff
    dstoff_col = np.full((NCORES, P, NBC * CC), 200.0, np.float32)
    jchunk = isB2 * CA + pos // P
    dstoff_col[core, pos % P, bl * CC + jchunk] = dstoff

    idxA = np.zeros((NCORES, LA), np.int16)
    mA = isB2 == 0
    idxA[core[mA], (bl * (CA * P) + pos)[mA]] = src2[mA].astype(np.int16)
    idxB = np.zeros((NCORES, LB), np.int16)
    mB = isB2 == 1
    idxB[core[mB], (bl * (CB * P) + pos)[mB]] = (src2[mB] - HALF).astype(np.int16)

    def wrap_idx(a):
        # [L] -> [128, L/16]: idx i at [i%16, i//16], replicated x8 partitions
        return np.tile(a.reshape(-1, 16).T, (8, 1)).copy()

    idxA_w = np.stack([wrap_idx(idxA[c]) for c in range(NCORES)])
    idxB_w = np.stack([wrap_idx(idxB[c]) for c in range(NCORES)])

    hT_aug = np.zeros((ATOM + 1, N_PAD), np.float32)
    hT_aug[:ATOM, :N_NODES] = h.T
    hT_aug[ATOM, :] = 1.0
    hT_own = hT_aug.reshape(ATOM + 1, NCORES, NPC).transpose(1, 0, 2).copy()

    n2gP = np.full(N_PAD, 999.0, np.float32)
    n2gP[:N_NODES] = n2g
    n2g_col = n2gP.reshape(NCORES, NBC, P).transpose(0, 2, 1).copy()

    WA = W_msg[0:ATOM].copy()                               # [64, 64]
    WBb = np.vstack([W_msg[ATOM:2 * ATOM], b_msg]).copy()   # [65, 64]
    W3 = W_msg[2 * ATOM:].copy()                            # [16, 64]
    WCb = np.vstack([W_upd[0:ATOM], b_upd]).copy()          # [65, 64]
    WD = W_upd[ATOM:].copy()                                # [64, 64]

    iota_f = np.broadcast_to(
        np.arange(P, dtype=np.float32), (P, P)).copy()
    iota_p = np.arange(P, dtype=np.float32).reshape(P, 1).copy()

    iden = np.eye(P, dtype=np.float32)
    common = dict(hT_aug=hT_aug, WA=WA, WBb=WBb, W3=W3, WCb=WCb, WD=WD,
                  iota_f=iota_f, iota_p=iota_p, iden=iden)
    in_maps = []
    for c in range(NCORES):
        m = dict(common)
        m.update(
            hT_own=hT_own[c],
            eT=eT[c],
            dstoff_col=dstoff_col[c],
            idxA=idxA_w[c],
            idxB=idxB_w[c],
            n2g_col=n2g_col[c],
        )
        in_maps.append(m)
    return in_maps, CA, CB


def build_nc(CA, CB):
    CC = CA + CB
    E_BLK = CC * P
    LA, LB = NBC * CA * P, NBC * CB * P

    nc = bacc.Bacc(None, target_bir_lowering=False, debug=False)

    hT_aug = nc.dram_tensor("hT_aug", [ATOM + 1, N_PAD], F32, kind="ExternalInput")
    hT_own = nc.dram_tensor("hT_own", [ATOM + 1, NPC], F32, kind="ExternalInput")
    eT_d = nc.dram_tensor("eT", [BOND, NBC * E_BLK], F32, kind="ExternalInput")
    doc_d = nc.dram_tensor("dstoff_col", [P, NBC * CC], F32, kind="ExternalInput")
    idxA_d = nc.dram_tensor("idxA", [P, LA // 16], I16, kind="ExternalInput")
    idxB_d = nc.dram_tensor("idxB", [P, LB // 16], I16, kind="ExternalInput")
    n2g_d = nc.dram_tensor("n2g_col", [P, NBC], F32, kind="ExternalInput")
    WA_d = nc.dram_tensor("WA", [ATOM, HDIM], F32, kind="ExternalInput")
    WBb_d = nc.dram_tensor("WBb", [ATOM + 1, HDIM], F32, kind="ExternalInput")
    W3_d = nc.dram_tensor("W3", [BOND, HDIM], F32, kind="ExternalInput")
    WCb_d = nc.dram_tensor("WCb", [ATOM + 1, HDIM], F32, kind="ExternalInput")
    WD_d = nc.dram_tensor("WD", [ATOM, HDIM], F32, kind="ExternalInput")
    iof_d = nc.dram_tensor("iota_f", [P, P], F32, kind="ExternalInput")
    iop_d = nc.dram_tensor("iota_p", [P, 1], F32, kind="ExternalInput")
    iden_d = nc.dram_tensor("iden", [P, P], F32, kind="ExternalInput")

    hnew_d = nc.dram_tensor("hnew", [NPC, HDIM], F32, kind="ExternalOutput")
    phis_d = nc.dram_tensor("phis_p", [NG, HDIM], F32, kind="ExternalOutput")

    P1_d = nc.dram_tensor("P1_tab", [N_PAD, HDIM], F32, kind="Internal")

    with tile.TileContext(nc) as tc:
        with tc.tile_pool(name="const", bufs=1) as cp, \
             tc.tile_pool(name="ht", bufs=2) as htp, \
             tc.tile_pool(name="stage", bufs=2) as stp, \
             tc.tile_pool(name="gout", bufs=2) as gp, \
             tc.tile_pool(name="etp", bufs=2) as etp, \
             tc.tile_pool(name="sgp", bufs=2) as sgp, \
             tc.tile_pool(name="msgp", bufs=2) as msgp, \
             tc.tile_pool(name="psum", bufs=1, space="PSUM") as pp:

            # ---- resident constants ----
            WAt = cp.tile([ATOM, HDIM], F32)
            nc.sync.dma_start(WAt[:], WA_d[:])
            WBbt = cp.tile([ATOM + 1, HDIM], F32)
            nc.sync.dma_start(WBbt[:], WBb_d[:])
            W3t = cp.tile([BOND, HDIM], F32)
            nc.sync.dma_start(W3t[:], W3_d[:])
            WCbt = cp.tile([ATOM + 1, HDIM], F32)
            nc.sync.dma_start(WCbt[:], WCb_d[:])
            WDt = cp.tile([ATOM, HDIM], F32)
            nc.sync.dma_start(WDt[:], WD_d[:])
            iof = cp.tile([P, P], F32)
            nc.sync.dma_start(iof[:], iof_d[:])
            iop = cp.tile([P, 1], F32)
            nc.sync.dma_start(iop[:], iop_d[:])
            iden_t = cp.tile([P, P], F32)
            nc.sync.dma_start(iden_t[:], iden_d[:])
            idxA_t = cp.tile([P, LA // 16], I16)
            nc.sync.dma_start(idxA_t[:], idxA_d[:])
            idxB_t = cp.tile([P, LB // 16], I16)
            nc.sync.dma_start(idxB_t[:], idxB_d[:])
            doc_t = cp.tile([P, NBC * CC], F32)
            nc.sync.dma_start(doc_t[:], doc_d[:])
            n2g_t = cp.tile([P, NBC], F32)
            nc.sync.dma_start(n2g_t[:], n2g_d[:])

            u_all = cp.tile([P, NBC * HDIM], F32)
            q_all = cp.tile([P, NBC * HDIM], F32)
            hnew_all = cp.tile([P, NBC * HDIM], F32)

            # ---- prologue: P1 = h @ WA (full), U/Q for own shard ----
            TN = 1024          # nodes per hT tile group (8 blocks of 128)
            n_p1_groups = N_PAD // TN   # 49
            for g in range(n_p1_groups):
                ht = htp.tile([ATOM + 1, TN], F32, tag="ht")
                nc.sync.dma_start(ht[:], hT_aug[:, g * TN:(g + 1) * TN])
                pm = pp.tile([P, 8 * HDIM], F32, space="PSUM", tag="pm8")
                for t in range(8):
                    nc.tensor.matmul(
                        out=pm[:, t * HDIM:(t + 1) * HDIM],
                        lhsT=ht[0:ATOM, t * P:(t + 1) * P],
                        rhs=WAt[:],
                        start=True, stop=True)
                st = stp.tile([P, 8 * HDIM], F32, tag="p1s")
                nc.scalar.copy(st[:], pm[:])
                # DRAM rows [g*TN, (g+1)*TN) viewed as [p, t, f]
                nc.sync.dma_start(
                    P1_d[g * TN:(g + 1) * TN, :].rearrange(
                        "(t p) f -> p t f", p=P),
                    st[:].rearrange("p (t f) -> p t f", f=HDIM))

            TO = 896           # own-shard tile: 7 blocks
            for g in range(NPC // TO):
                hto = htp.tile([ATOM + 1, TO], F32, tag="hto")
                nc.sync.dma_start(hto[:], hT_own[:, g * TO:(g + 1) * TO])
                for t in range(7):
                    b = g * 7 + t
                    pu = pp.tile([P, HDIM], F32, space="PSUM", tag="pm1")
                    nc.tensor.matmul(
                        out=pu[:], lhsT=hto[:, t * P:(t + 1) * P],
                        rhs=WBbt[:], start=True, stop=True)
                    nc.scalar.copy(u_all[:, b * HDIM:(b + 1) * HDIM], pu[:])
                    pq = pp.tile([P, HDIM], F32, space="PSUM", tag="pm1")
                    nc.tensor.matmul(
                        out=pq[:], lhsT=hto[:, t * P:(t + 1) * P],
                        rhs=WCbt[:], start=True, stop=True)
                    nc.scalar.copy(q_all[:, b * HDIM:(b + 1) * HDIM], pq[:])

            # ---- main loop over node blocks ----
            if NBC_RUN < NBC:
                nc.gpsimd.memset(hnew_all[:], 0.0)
            pphis = pp.tile([NG, HDIM], F32, space="PSUM", tag="pphis")
            goutA = goutB = None
            for b in range(NBC_RUN):
                g, bi = divmod(b, GG)
                gblocks = min(GG, NBC_RUN - g * GG, NBC - g * GG)
                if bi == 0:
                    nA = gblocks * CA * P
                    goutA = gp.tile([P, GG * CA, HDIM], F32, tag="goutA")
                    nB = gblocks * CB * P
                    goutB = gp.tile([P, GG * CB, HDIM], F32, tag="goutB")
                    for (gout, tab, idx_t, ntot, base) in (
                            (goutA, P1_d[0:HALF, :], idxA_t, nA, g * GG * CA * P),
                            (goutB, P1_d[HALF:, :], idxB_t, nB, g * GG * CB * P)):
                        off = 0
                        while off < ntot:
                            n = min(GCALL, ntot - off)
                            nc.gpsimd.dma_gather(
                                out_ap=gout[:, off // P:(off + n) // P, :],
                                in_ap=tab,
                                idxs_ap=idx_t[:, (base + off) // 16:
                                              (base + off + n) // 16],
                                num_idxs=n, num_idxs_reg=n, elem_size=HDIM)
                            off += n

                et = etp.tile([BOND, E_BLK], F32, tag="et")
                nc.sync.dma_start(et[:], eT_d[:, b * E_BLK:(b + 1) * E_BLK])

                # S_gT [128n, E_BLK] = (dstoff_e == n): per 128-edge chunk,
                # PE-transpose the per-partition dstoff column into a
                # row-replicated [128, 128] psum block, then DVE-compare
                # against the partition index (iota_p) per 512-wide piece.
                sgt = sgp.tile([P, E_BLK], F32, tag="sgt")
                for piece in range(_ceil_div(CC, 4)):
                    j0 = piece * 4
                    nch = min(4, CC - j0)
                    pbc = pp.tile([P, 4 * P], F32, space="PSUM", tag="pbc")
                    for ci in range(nch):
                        col = b * CC + j0 + ci
                        nc.tensor.transpose(
                            out=pbc[:, ci * P:(ci + 1) * P],
                            in_=doc_t[:, col:col + 1].to_broadcast([P, P]),
                            identity=iden_t[:])
                    nc.vector.tensor_scalar(
                        out=sgt[:, j0 * P:(j0 + nch) * P],
                        in0=pbc[:, 0:nch * P],
                        scalar1=iop[:, 0:1], scalar2=None,
                        op0=mybir.AluOpType.is_equal)

                u_b = u_all[:, b * HDIM:(b + 1) * HDIM]

                # chunk groups: A run split (8, CA-8), B run split (8, CB-8)
                groups = []
                j0 = 0
                for run_len in (CA, CB):
                    take = 0
                    while take < run_len:
                        w = min(8, run_len - take)
                        groups.append((j0 + take, w))
                        take += w
                    j0 += run_len
                paggT = pp.tile([HDIM, P], F32, space="PSUM", tag="paggT")
                for (jg, wg) in groups:
                    pm = pp.tile([P, wg * HDIM], F32, space="PSUM",
                                 tag=("pm8" if wg > 4 else "pm1"))
                    for r in range(wg):
                        j = jg + r
                        nc.tensor.matmul(
                            out=pm[:, r * HDIM:(r + 1) * HDIM],
                            lhsT=sgt[:, j * P:(j + 1) * P],
                            rhs=u_b,
                            start=True, stop=False)
                        nc.tensor.matmul(
                            out=pm[:, r * HDIM:(r + 1) * HDIM],
                            lhsT=et[:, j * P:(j + 1) * P],
                            rhs=W3t[:],
                            start=False, stop=True)
                    # msg = relu(pm + gathered P1[src])
                    if jg < CA:
                        gsl = goutA[:, bi * CA + jg: bi * CA + jg + wg, :]
                    else:
                        gsl = goutB[:, bi * CB + (jg - CA):
                                    bi * CB + (jg - CA) + wg, :]
                    msg = msgp.tile([P, wg * HDIM], F32,
                                    tag=("msg8" if wg > 4 else "msg1"))
                    nc.vector.tensor_tensor(
                        out=msg[:], in0=pm[:],
                        in1=gsl.rearrange("p a b -> p (a b)"),
                        op=mybir.AluOpType.add)
                    nc.scalar.activation(
                        msg[:], msg[:], mybir.ActivationFunctionType.Relu)
                    for r in range(wg):
                        j = jg + r
                        sg = sgp.tile([P, P], F32, tag="sg")
                        nc.vector.tensor_scalar(
                            out=sg[:], in0=iof[:],
                            scalar1=doc_t[:, b * CC + j: b * CC + j + 1],
                            scalar2=None, op0=mybir.AluOpType.is_equal)
                        nc.tensor.matmul(
                            out=paggT[:],
                            lhsT=msg[:, r * HDIM:(r + 1) * HDIM],
                            rhs=sg[:],
                            start=(j == 0), stop=(j == CC - 1))

                # node update
                aggT_s = stp.tile([HDIM, P], F32, tag="aggT")
                nc.scalar.copy(aggT_s[:], paggT[:])
                phn = pp.tile([P, HDIM], F32, space="PSUM", tag="phn")
                nc.tensor.matmul(out=phn[:], lhsT=aggT_s[:], rhs=WDt[:],
                                 start=True, stop=True)
                hsl = hnew_all[:, b * HDIM:(b + 1) * HDIM]
                nc.vector.tensor_tensor(
                    out=hsl, in0=phn[:], in1=q_all[:, b * HDIM:(b + 1) * HDIM],
                    op=mybir.AluOpType.add)
                nc.scalar.activation(
                    hsl, hsl, mybir.ActivationFunctionType.Relu)

                # readout
                gb = sgp.tile([P, P], F32, tag="gb")
                nc.vector.tensor_scalar(
                    out=gb[:], in0=iof[:], scalar1=n2g_t[:, b:b + 1],
                    scalar2=None, op0=mybir.AluOpType.is_equal)
                nc.tensor.matmul(out=pphis[:], lhsT=gb[:], rhs=hsl,
                                 start=(b == 0), stop=(b == NBC_RUN - 1))

            # ---- epilogue ----
            phis_s = stp.tile([NG, HDIM], F32, tag="phis")
            if NBC_RUN > 0:
                nc.scalar.copy(phis_s[:], pphis[:])
            else:
                nc.gpsimd.memset(phis_s[:], 0.0)
            nc.sync.dma_start(phis_d[:], phis_s[:])
            nc.sync.dma_start(
                hnew_d[:].rearrange("(b p) f -> p b f", p=P),
                hnew_all[:].rearrange("p (b f) -> p b f", f=HDIM))

    return nc


def kernel(**inputs):
    from concourse.bass_utils import run_bass_kernel_spmd

    in_maps, CA, CB = _preprocess(**inputs)
    nc = build_nc(CA, CB)
    nc.finalize()
    res = run_bass_kernel_spmd(nc, in_maps, core_ids=list(range(NCORES)),
                               **RUN_KWARGS)
    outs = res.results
    h_new = np.concatenate([outs[c]["hnew"] for c in range(NCORES)],
                           axis=0)[:N_NODES]
    phis = np.sum([outs[c]["phis_p"] for c in range(NCORES)], axis=0,
                  dtype=np.float32).astype(np.float32)
    kernel.last_results = res
    return h_new, phis


kernel.last_results = None


# revision 13
# speedup vs baseline: 2.0316x; 2.0316x over previous
"""Trainium2 Bass kernel for Battaglia-style GNN message passing (SPMD, 8 cores).

Problem:
    edge_in = concat(h[src], h[dst], e)            # [E, 144]
    msg     = relu(edge_in @ W_msg + b_msg)        # [E, 64]
    agg     = segment_sum(msg, dst, N)             # [N, 64]
    h_new   = relu(concat(h, agg) @ W_upd + b_upd) # [N, 64]
    phis    = segment_sum(h_new, node2graph, G)    # [G, 64]

Strategy (dst-sharded edge-parallel):
  * Host sorts edges by dst, shards nodes (and their incoming edges)
    contiguously across 8 cores; 128-node blocks; per-block edge runs are
    split by src < HALF (int16 gather-index range) into A/B runs, each
    padded to a uniform chunk count (CA/CB chunks of 128 edges).
  * Algebraic folding: W_msg = [W1; W2; W3] rows for (h_src, h_dst, e).
    Device precomputes P1 = h@W1 (full, to DRAM gather table),
    U = h@W2 + b_msg and Q = h@Wu1 + b_upd for own node shard (SBUF).
  * Per edge chunk [128e]:  psum = S_gT.T @ u_block + eT.T @ W3;
    msg = relu(psum + P1[src]) where P1[src] comes from bulk dma_gather;
    aggT[64, 128n] += msg.T-matmul with one-hot S_g (the scatter).
    One-hots built on DVE via iota/is_equal; S_gT via partition_broadcast.
  * Node update + graph readout per block via small matmuls; per-core
    partial phis summed on host; h_new shards concatenated on host.
"""
import sys

for _p in ("/opt/trn_rl_repo",):
    if _p not in sys.path:
        sys.path.insert(0, _p)

import numpy as np
import ml_dtypes

from concourse import bacc, mybir
import concourse.tile as tile

P = 128
N_NODES = 50000
N_EDGES = 800000
ATOM = 64
BOND = 16
HDIM = 64
NG = 128
NCORES = 8

NB = 392                  # node blocks total (N padded to 50176)
N_PAD = NB * P            # 50176
NBC = NB // NCORES        # 49 blocks per core
NPC = NBC * P             # 6272 nodes per core
HALF = N_PAD // 2         # 25088: int16 gather table split

F32 = mybir.dt.float32
BF16 = mybir.dt.bfloat16
I16 = mybir.dt.int16

# knobs
GG = 4                    # blocks per gather output-tile group
GCALL = 1024              # max idxs per dma_gather call (HW-validated limit)
NBC_RUN = NBC             # blocks actually processed (bisection knob)
FEAT_OFF = set()          # feature-bisection: names to disable

# set by test harness for profiling
RUN_KWARGS = {}


def _ceil_div(a, b):
    return (a + b - 1) // b


def _preprocess(h, e, W_msg, b_msg, W_upd, b_upd, src, dst, node2graph):
    h = np.asarray(h, np.float32)
    e = np.asarray(e, np.float32)
    W_msg = np.asarray(W_msg, np.float32)
    b_msg = np.asarray(b_msg, np.float32)
    W_upd = np.asarray(W_upd, np.float32)
    b_upd = np.asarray(b_upd, np.float32)
    src = np.asarray(src, np.int64)
    dst = np.asarray(dst, np.int64)
    n2g = np.asarray(node2graph, np.int64)

    E = len(src)
    order = np.argsort(dst, kind="stable")
    src_s = src[order]
    dst_s = dst[order]
    e_s = e[order]

    blk = dst_s // P
    isB = (src_s >= HALF).astype(np.int64)
    key = blk * 2 + isB
    order2 = np.argsort(key, kind="stable")
    src2 = src_s[order2]
    dst2 = dst_s[order2]
    e2 = e_s[order2]
    key2 = key[order2]
    blk2 = blk[order2]
    isB2 = isB[order2]

    cnt = np.bincount(key2, minlength=2 * NB)
    CA = max(1, _ceil_div(int(cnt[0::2].max()), P))
    CB = max(1, _ceil_div(int(cnt[1::2].max()), P))
    CC = CA + CB
    E_BLK = CC * P

    starts = np.zeros(2 * NB, np.int64)
    starts[1:] = np.cumsum(cnt)[:-1]
    pos = np.arange(E, dtype=np.int64) - starts[key2]

    core = blk2 // NBC
    bl = blk2 % NBC                    # block within core
    slot = bl * E_BLK + isB2 * (CA * P) + pos   # slot within core edge array
    dstoff = (dst2 - blk2 * P).astype(np.float32)

    LA, LB = NBC * CA * P, NBC * CB * P

    eT = np.zeros((NCORES, BOND, NBC * E_BLK), ml_dtypes.bfloat16)
    eT[core, :, slot] = e2.astype(ml_dtypes.bfloat16)
    # host-built one-hot matrices (bf16, exact 0/1):
    # S_g_all [128e, NBC*CC*128n]: per chunk j, S_g[e, n] = (dstoff == n)
    # S_gT_all [128n, NBC*E_BLK e]: per block, S_gT[n, e] = (dstoff_e == n)
    jchunk = isB2 * CA + pos // P
    sg_all = np.zeros((NCORES, P, NBC * CC * P), ml_dtypes.bfloat16)
    sg_all[core, pos % P, (bl * CC + jchunk) * P + dstoff.astype(np.int64)] = 1
    sgt_all = np.zeros((NCORES, P, NBC * E_BLK), ml_dtypes.bfloat16)
    sgt_all[core, dstoff.astype(np.int64), bl * E_BLK + isB2 * (CA * P) + pos] = 1

    idxA = np.zeros((NCORES, LA), np.int16)
    mA = isB2 == 0
    idxA[core[mA], (bl * (CA * P) + pos)[mA]] = src2[mA].astype(np.int16)
    idxB = np.zeros((NCORES, LB), np.int16)
    mB = isB2 == 1
    idxB[core[mB], (bl * (CB * P) + pos)[mB]] = (src2[mB] - HALF).astype(np.int16)

    def wrap_idx(a):
        # [L] -> [128, L/16]: idx i at [i%16, i//16], replicated x8 partitions
        return np.tile(a.reshape(-1, 16).T, (8, 1)).copy()

    idxA_w = np.stack([wrap_idx(idxA[c]) for c in range(NCORES)])
    idxB_w = np.stack([wrap_idx(idxB[c]) for c in range(NCORES)])

    hT_aug = np.zeros((ATOM + 1, N_PAD), np.float32)
    hT_aug[:ATOM, :N_NODES] = h.T
    hT_aug[ATOM, :] = 1.0
    hT_own = hT_aug.reshape(ATOM + 1, NCORES, NPC).transpose(1, 0, 2).copy()

    n2gP = np.full(N_PAD, 999.0, np.float32)
    n2gP[:N_NODES] = n2g
    n2g_col = n2gP.reshape(NCORES, NBC, P).transpose(0, 2, 1).copy()

    WA = W_msg[0:ATOM].copy()                               # [64, 64]
    WBb = np.vstack([W_msg[ATOM:2 * ATOM], b_msg]).copy()   # [65, 64]
    W3 = W_msg[2 * ATOM:].astype(ml_dtypes.bfloat16)        # [16, 64]
    WCb = np.vstack([W_upd[0:ATOM], b_upd]).copy()          # [65, 64]
    WD = W_upd[ATOM:].astype(ml_dtypes.bfloat16)            # [64, 64]

    iota_f = np.broadcast_to(
        np.arange(P, dtype=np.float32), (P, P)).copy()

    common = dict(hT_aug=hT_aug, WA=WA, WBb=WBb, W3=W3, WCb=WCb, WD=WD,
                  iota_f=iota_f)
    in_maps = []
    for c in range(NCORES):
        m = dict(common)
        m.update(
            hT_own=hT_own[c],
            eT=eT[c],
            sg_all=sg_all[c],
            sgt_all=sgt_all[c],
            idxA=idxA_w[c],
            idxB=idxB_w[c],
            n2g_col=n2g_col[c],
        )
        in_maps.append(m)
    return in_maps, CA, CB


def build_nc(CA, CB):
    CC = CA + CB
    E_BLK = CC * P
    LA, LB = NBC * CA * P, NBC * CB * P

    nc = bacc.Bacc(None, target_bir_lowering=False, debug=False)

    hT_aug = nc.dram_tensor("hT_aug", [ATOM + 1, N_PAD], F32, kind="ExternalInput")
    hT_own = nc.dram_tensor("hT_own", [ATOM + 1, NPC], F32, kind="ExternalInput")
    eT_d = nc.dram_tensor("eT", [BOND, NBC * E_BLK], BF16, kind="ExternalInput")
    sg_d = nc.dram_tensor("sg_all", [P, NBC * CC * P], BF16, kind="ExternalInput")
    sgt_d = nc.dram_tensor("sgt_all", [P, NBC * E_BLK], BF16, kind="ExternalInput")
    idxA_d = nc.dram_tensor("idxA", [P, LA // 16], I16, kind="ExternalInput")
    idxB_d = nc.dram_tensor("idxB", [P, LB // 16], I16, kind="ExternalInput")
    n2g_d = nc.dram_tensor("n2g_col", [P, NBC], F32, kind="ExternalInput")
    WA_d = nc.dram_tensor("WA", [ATOM, HDIM], F32, kind="ExternalInput")
    WBb_d = nc.dram_tensor("WBb", [ATOM + 1, HDIM], F32, kind="ExternalInput")
    W3_d = nc.dram_tensor("W3", [BOND, HDIM], BF16, kind="ExternalInput")
    WCb_d = nc.dram_tensor("WCb", [ATOM + 1, HDIM], F32, kind="ExternalInput")
    WD_d = nc.dram_tensor("WD", [ATOM, HDIM], BF16, kind="ExternalInput")
    iof_d = nc.dram_tensor("iota_f", [P, P], F32, kind="ExternalInput")

    hnew_d = nc.dram_tensor("hnew", [NPC, HDIM], F32, kind="ExternalOutput")
    phis_d = nc.dram_tensor("phis_p", [NG, HDIM], F32, kind="ExternalOutput")

    P1_d = nc.dram_tensor("P1_tab", [N_PAD, HDIM], F32, kind="Internal")

    with tile.TileContext(nc) as tc:
        with tc.tile_pool(name="const", bufs=1) as cp, \
             tc.tile_pool(name="ht", bufs=2) as htp, \
             tc.tile_pool(name="stage", bufs=2) as stp, \
             tc.tile_pool(name="gout", bufs=2) as gp, \
             tc.tile_pool(name="etp", bufs=2) as etp, \
             tc.tile_pool(name="sgp", bufs=2) as sgp, \
             tc.tile_pool(name="msgp", bufs=2) as msgp, \
             tc.tile_pool(name="psum", bufs=2, space="PSUM") as pp, \
             tc.tile_pool(name="psum1", bufs=1, space="PSUM") as pp1:

            # ---- resident constants ----
            WAt = cp.tile([ATOM, HDIM], F32)
            nc.sync.dma_start(WAt[:], WA_d[:])
            WBbt = cp.tile([ATOM + 1, HDIM], F32)
            nc.sync.dma_start(WBbt[:], WBb_d[:])
            W3t = cp.tile([BOND, HDIM], BF16)
            nc.sync.dma_start(W3t[:], W3_d[:])
            WCbt = cp.tile([ATOM + 1, HDIM], F32)
            nc.sync.dma_start(WCbt[:], WCb_d[:])
            WDt = cp.tile([ATOM, HDIM], BF16)
            nc.sync.dma_start(WDt[:], WD_d[:])
            iof = cp.tile([P, P], F32)
            nc.sync.dma_start(iof[:], iof_d[:])
            idxA_t = cp.tile([P, LA // 16], I16)
            nc.sync.dma_start(idxA_t[:], idxA_d[:])
            idxB_t = cp.tile([P, LB // 16], I16)
            nc.sync.dma_start(idxB_t[:], idxB_d[:])

            n2g_t = cp.tile([P, NBC], F32)
            nc.sync.dma_start(n2g_t[:], n2g_d[:])

            u_all = cp.tile([P, NBC * HDIM], BF16)
            q_all = cp.tile([P, NBC * HDIM], F32)
            hnew_all = cp.tile([P, NBC * HDIM], F32)

            # ---- prologue: P1 = h @ WA (full), U/Q for own shard ----
            TN = 1024          # nodes per hT tile group (8 blocks of 128)
            n_p1_groups = N_PAD // TN   # 49
            for g in range(n_p1_groups):
                ht = htp.tile([ATOM + 1, TN], F32, tag="ht")
                nc.sync.dma_start(ht[:], hT_aug[:, g * TN:(g + 1) * TN])
                pm = pp.tile([P, 8 * HDIM], F32, space="PSUM", tag="pm8")
                for t in range(8):
                    nc.tensor.matmul(
                        out=pm[:, t * HDIM:(t + 1) * HDIM],
                        lhsT=ht[0:ATOM, t * P:(t + 1) * P],
                        rhs=WAt[:],
                        start=True, stop=True)
                st = stp.tile([P, 8 * HDIM], F32, tag="p1s")
                nc.scalar.copy(st[:], pm[:])
                # DRAM rows [g*TN, (g+1)*TN) viewed as [p, t, f]
                nc.sync.dma_start(
                    P1_d[g * TN:(g + 1) * TN, :].rearrange(
                        "(t p) f -> p t f", p=P),
                    st[:].rearrange("p (t f) -> p t f", f=HDIM))

            TO = 896           # own-shard tile: 7 blocks
            for g in range(NPC // TO):
                hto = htp.tile([ATOM + 1, TO], F32, tag="hto")
                nc.sync.dma_start(hto[:], hT_own[:, g * TO:(g + 1) * TO])
                for t in range(7):
                    b = g * 7 + t
                    pu = pp.tile([P, HDIM], F32, space="PSUM", tag="pm1")
                    nc.tensor.matmul(
                        out=pu[:], lhsT=hto[:, t * P:(t + 1) * P],
                        rhs=WBbt[:], start=True, stop=True)
                    nc.scalar.copy(u_all[:, b * HDIM:(b + 1) * HDIM], pu[:])
                    pq = pp.tile([P, HDIM], F32, space="PSUM", tag="pm1")
                    nc.tensor.matmul(
                        out=pq[:], lhsT=hto[:, t * P:(t + 1) * P],
                        rhs=WCbt[:], start=True, stop=True)
                    nc.scalar.copy(q_all[:, b * HDIM:(b + 1) * HDIM], pq[:])

            # ---- main loop over node blocks ----
            if NBC_RUN < NBC:
                nc.gpsimd.memset(hnew_all[:], 0.0)
            pphis = pp1.tile([NG, HDIM], F32, space="PSUM", tag="pphis")
            goutA = goutB = None
            for b in range(NBC_RUN):
                g, bi = divmod(b, GG)
                gblocks = min(GG, NBC_RUN - g * GG, NBC - g * GG)
                if bi == 0:
                    nA = gblocks * CA * P
                    goutA = gp.tile([P, GG * CA, HDIM], F32, tag="goutA")
                    nB = gblocks * CB * P
                    goutB = gp.tile([P, GG * CB, HDIM], F32, tag="goutB")
                    for (gout, tab, idx_t, ntot, base) in (
                            (goutA, P1_d[0:HALF, :], idxA_t, nA, g * GG * CA * P),
                            (goutB, P1_d[HALF:, :], idxB_t, nB, g * GG * CB * P)):
                        off = 0
                        while off < ntot:
                            n = min(GCALL, ntot - off)
                            nc.gpsimd.dma_gather(
                                out_ap=gout[:, off // P:(off + n) // P, :],
                                in_ap=tab,
                                idxs_ap=idx_t[:, (base + off) // 16:
                                              (base + off + n) // 16],
                                num_idxs=n, num_idxs_reg=n, elem_size=HDIM)
                            off += n

                et = etp.tile([BOND, E_BLK], BF16, tag="et")
                nc.sync.dma_start(et[:], eT_d[:, b * E_BLK:(b + 1) * E_BLK])
                sgt = sgp.tile([P, E_BLK], BF16, tag="sgt")
                nc.sync.dma_start(sgt[:], sgt_d[:, b * E_BLK:(b + 1) * E_BLK])
                sga = sgp.tile([P, CC * P], BF16, tag="sga")
                nc.sync.dma_start(sga[:], sg_d[:, b * CC * P:(b + 1) * CC * P])

                u_b = u_all[:, b * HDIM:(b + 1) * HDIM]

                # chunk groups: A run split (8, CA-8), B run split (8, CB-8)
                groups = []
                j0 = 0
                for run_len in (CA, CB):
                    take = 0
                    while take < run_len:
                        w = min(8, run_len - take)
                        groups.append((j0 + take, w))
                        take += w
                    j0 += run_len
                paggT = pp.tile([HDIM, P], F32, space="PSUM", tag="paggT")
                for (jg, wg) in groups:
                    pm = pp.tile([P, wg * HDIM], F32, space="PSUM",
                                 tag=("pm8" if wg > 4 else "pm1"))
                    for r in range(wg):
                        j = jg + r
                        nc.tensor.matmul(
                            out=pm[:, r * HDIM:(r + 1) * HDIM],
                            lhsT=sgt[:, j * P:(j + 1) * P],
                            rhs=u_b,
                            start=True, stop=False)
                        nc.tensor.matmul(
                            out=pm[:, r * HDIM:(r + 1) * HDIM],
                            lhsT=et[:, j * P:(j + 1) * P],
                            rhs=W3t[:],
                            start=False, stop=True)
                    # msg = relu(pm + gathered P1[src])
                    if jg < CA:
                        gsl = goutA[:, bi * CA + jg: bi * CA + jg + wg, :]
                    else:
                        gsl = goutB[:, bi * CB + (jg - CA):
                                    bi * CB + (jg - CA) + wg, :]
                    msgf = msgp.tile([P, wg * HDIM], F32,
                                     tag=("msgf8" if wg > 4 else "msgf1"))
                    nc.vector.tensor_tensor(
                        out=msgf[:], in0=pm[:],
                        in1=gsl.rearrange("p a b -> p (a b)"),
                        op=mybir.AluOpType.add)
                    msg = msgp.tile([P, wg * HDIM], BF16,
                                    tag=("msg8" if wg > 4 else "msg1"))
                    nc.scalar.activation(
                        msg[:], msgf[:], mybir.ActivationFunctionType.Relu)
                    for r in range(wg):
                        j = jg + r
                        nc.tensor.matmul(
                            out=paggT[:],
                            lhsT=msg[:, r * HDIM:(r + 1) * HDIM],
                            rhs=sga[:, j * P:(j + 1) * P],
                            start=(j == 0), stop=(j == CC - 1))

                # node update
                aggT_s = stp.tile([HDIM, P], BF16, tag="aggT")
                nc.scalar.copy(aggT_s[:], paggT[:])
                phn = pp1.tile([P, HDIM], F32, space="PSUM", tag="phn")
                nc.tensor.matmul(out=phn[:], lhsT=aggT_s[:], rhs=WDt[:],
                                 start=True, stop=True)
                hsl = hnew_all[:, b * HDIM:(b + 1) * HDIM]
                nc.vector.tensor_tensor(
                    out=hsl, in0=phn[:], in1=q_all[:, b * HDIM:(b + 1) * HDIM],
                    op=mybir.AluOpType.add)
                nc.scalar.activation(
                    hsl, hsl, mybir.ActivationFunctionType.Relu)

                # readout
                gb = sgp.tile([P, P], F32, tag="gb")
                nc.vector.tensor_scalar(
                    out=gb[:], in0=iof[:], scalar1=n2g_t[:, b:b + 1],
                    scalar2=None, op0=mybir.AluOpType.is_equal)
                nc.tensor.matmul(out=pphis[:], lhsT=gb[:], rhs=hsl,
                                 start=(b == 0), stop=(b == NBC_RUN - 1))

            # ---- epilogue ----
            phis_s = stp.tile([NG, HDIM], F32, tag="phis")
            if NBC_RUN > 0:
                nc.scalar.copy(phis_s[:], pphis[:])
            else:
                nc.gpsimd.memset(phis_s[:], 0.0)
            nc.sync.dma_start(phis_d[:], phis_s[:])
            nc.sync.dma_start(
                hnew_d[:].rearrange("(b p) f -> p b f", p=P),
                hnew_all[:].rearrange("p (b f) -> p b f", f=HDIM))

    return nc


def kernel(**inputs):
    from concourse.bass_utils import run_bass_kernel_spmd

    in_maps, CA, CB = _preprocess(**inputs)
    nc = build_nc(CA, CB)
    nc.finalize()
    res = run_bass_kernel_spmd(nc, in_maps, core_ids=list(range(NCORES)),
                               **RUN_KWARGS)
    outs = res.results
    h_new = np.concatenate([outs[c]["hnew"] for c in range(NCORES)],
                           axis=0)[:N_NODES]
    phis = np.sum([outs[c]["phis_p"] for c in range(NCORES)], axis=0,
                  dtype=np.float32).astype(np.float32)
    kernel.last_results = res
    return h_new, phis


kernel.last_results = None
